# revision 16
# baseline (speedup 1.0000x reference)
"""Trainium2 Bass kernel for the Context Encoder problem:

    ce  = c2e_weight[nodes]            # [N, 128] embedding gather
    h   = relu(ce @ w1.T + b1)         # [N, 128]
    out = relu(h @ w2.T + b2)          # [N, 128]

Strategy (8 NeuronCores, vocab-range sharding):
  200000 node ids over a 100000-row vocab saturate every vocab window,
  so transforming the table itself is less work than gathering per-node
  rows (and avoids the per-index DMA descriptor-generation cost that
  dominates any on-device gather).

  - The vocab is split into 8 fixed 12500-row ranges.  Core i streams
    its host-pre-transposed (d-major) table window [128, 12800]
    contiguously at full DMA bandwidth and computes
    T2 = relu(relu(win @ w1.T + b1) @ w2.T + b2) for every window row.
  - d-major input feeds mm1 directly (lhsT = w1.T stationary, window as
    the moving operand); mm2 keeps w2.T stationary.  Both layers' biases
    are per-partition, so relu+bias fuses into one ScalarE activation or
    one VectorE dual-op tensor_scalar; the two relus alternate between
    ACT and DVE for engine balance.  No PE transposes, no PSUM staging
    copies, no bias matmuls.
  - Results stay feature-major; the host transposes each window and maps
    node positions to rows (out = T2[nodes]) as the unshard step.
"""

import os
import sys

for _p in ("/opt/trn_rl_repo",):
    if _p not in sys.path:
        sys.path.insert(0, _p)

import numpy as np

import concourse.bass as bass
import concourse.mybir as mybir
import concourse.tile as tile
from concourse import bacc
from concourse.bass_utils import run_bass_kernel_spmd
from concourse.tile import TileContext

P = 128
D = 128
N_CORES = 8
VOCAB = 100000
RANGE = VOCAB // N_CORES   # 12500 vocab rows owned per core
BLOCKS = 100               # 12800 rows processed per core (128*100)
CHUNK_BLOCKS = 20          # rows DMA'd per input chunk (1.31 MB)
G = 4                      # blocks per compute super-tile (free dim 512)


def build_nc(blocks: int = BLOCKS, chunk_blocks: int = CHUNK_BLOCKS,
             g: int = G, use_f32r: bool = False):
    assert blocks % g == 0 and chunk_blocks % g == 0
    f32 = mybir.dt.float32
    nc = bacc.Bacc("TRN2", target_bir_lowering=False, debug=False,
                   num_devices=N_CORES)

    rows = blocks * P
    tsl_t = nc.dram_tensor("tslice", [P, rows], f32,
                           kind="ExternalInput").ap()
    w1t_t = nc.dram_tensor("w1t", [D, D], f32, kind="ExternalInput").ap()
    w2t_t = nc.dram_tensor("w2t", [D, D], f32, kind="ExternalInput").ap()
    b1_t = nc.dram_tensor("b1c", [P, 1], f32, kind="ExternalInput").ap()
    b2_t = nc.dram_tensor("b2c", [P, 1], f32, kind="ExternalInput").ap()
    out_t = nc.dram_tensor("out", [P, rows], f32,
                           kind="ExternalOutput").ap()

    fw = g * D  # super-tile free width (512)

    with TileContext(nc) as tc:
        with (
            tc.tile_pool(name="const", bufs=1) as cpool,
            tc.tile_pool(name="win", bufs=3) as gpool,
            tc.tile_pool(name="work", bufs=3) as wpool,
            tc.tile_pool(name="psum", bufs=3, space="PSUM") as ppool,
        ):
            # consts go on the gpsimd SWDGE queue so the first window
            # chunk heads the sync-engine HWDGE FIFO
            w1t_sb = cpool.tile([D, D], f32, tag="w1t")
            nc.gpsimd.dma_start(out=w1t_sb[:], in_=w1t_t[:])
            w2t_sb = cpool.tile([D, D], f32, tag="w2t")
            nc.gpsimd.dma_start(out=w2t_sb[:], in_=w2t_t[:])
            b1_sb = cpool.tile([P, 1], f32, tag="b1")
            nc.gpsimd.dma_start(out=b1_sb[:], in_=b1_t[:])
            b2_sb = cpool.tile([P, 1], f32, tag="b2")
            nc.gpsimd.dma_start(out=b2_sb[:], in_=b2_t[:])

            def relu_bias(out_ap, in_ap, bias_sb, on_act: bool):
                if on_act:
                    nc.scalar.activation(out_ap, in_ap,
                                         mybir.ActivationFunctionType.Relu,
                                         bias=bias_sb[:, 0:1])
                else:
                    nc.vector.tensor_scalar(
                        out=out_ap, in0=in_ap, scalar1=bias_sb[:, 0:1],
                        scalar2=0.0, op0=mybir.AluOpType.add,
                        op1=mybir.AluOpType.max)

            def mmcast(ap):
                return ap.bitcast(mybir.dt.float32r) if use_f32r else ap

            # small first chunk so mm1 starts as early as possible
            chunks = [g] + [chunk_blocks] * ((blocks - g) // chunk_blocks)
            rem = blocks - sum(chunks)
            assert rem % g == 0
            if rem:
                chunks.append(rem)

            st = 0
            r0 = 0
            for cb in chunks:
                win = gpool.tile([P, chunk_blocks * D], f32, tag="win")
                nc.sync.dma_start(
                    out=win[:, : cb * D], in_=tsl_t[:, r0 : r0 + cb * P])
                for s in range(cb // g):
                    r0s = r0 + s * fw
                    ceT = win[:, s * fw : (s + 1) * fw]

                    h_ps = ppool.tile([P, fw], f32, tag="h")
                    nc.tensor.matmul(out=h_ps[:], lhsT=mmcast(w1t_sb[:]),
                                     rhs=mmcast(ceT), start=True, stop=True)
                    hT_sb = wpool.tile([P, fw], f32, tag="hT")
                    relu_bias(hT_sb[:], h_ps[:], b1_sb, on_act=(st % 2 == 0))

                    o_ps = ppool.tile([P, fw], f32, tag="o")
                    nc.tensor.matmul(out=o_ps[:], lhsT=mmcast(w2t_sb[:]),
                                     rhs=mmcast(hT_sb[:]), start=True,
                                     stop=True)
                    o_sb = wpool.tile([P, fw], f32, tag="o_sb")
                    relu_bias(o_sb[:], o_ps[:], b2_sb, on_act=(st % 2 == 1))
                    st += 1

                    nc.sync.dma_start(out=out_t[:, r0s : r0s + fw],
                                      in_=o_sb[:])
                r0 += cb * P

    nc.compile()
    return nc


_CACHED_NC = None
LAST_RESULTS = None


def _get_nc():
    global _CACHED_NC
    if _CACHED_NC is None:
        _CACHED_NC = build_nc(
            use_f32r=os.environ.get("BASS_KERNEL_F32R", "0") == "1")
    return _CACHED_NC


def kernel(nodes, c2e_weight, w1, b1, w2, b2):
    nodes = np.asarray(nodes).astype(np.int64)
    c2e_weight = np.asarray(c2e_weight, dtype=np.float32)
    w1 = np.asarray(w1, dtype=np.float32)
    b1 = np.asarray(b1, dtype=np.float32)
    w2 = np.asarray(w2, dtype=np.float32)
    b2 = np.asarray(b2, dtype=np.float32)

    vocab = c2e_weight.shape[0]
    assert vocab == VOCAB, vocab
    rows = BLOCKS * P  # 12800

    tableT = np.ascontiguousarray(c2e_weight.T)  # [128, VOCAB], d-major

    w1t = np.ascontiguousarray(w1.T)
    w2t = np.ascontiguousarray(w2.T)
    b1c = np.ascontiguousarray(b1.reshape(P, 1))
    b2c = np.ascontiguousarray(b2.reshape(P, 1))

    starts = []
    in_maps = []
    for i in range(N_CORES):
        start = min(i * RANGE, vocab - rows)
        starts.append(start)
        in_maps.append({
            "tslice": np.ascontiguousarray(tableT[:, start : start + rows]),
            "w1t": w1t,
            "w2t": w2t,
            "b1c": b1c,
            "b2c": b2c,
        })

    nc = _get_nc()
    trace = os.environ.get("BASS_KERNEL_TRACE") == "1"
    res = run_bass_kernel_spmd(nc, in_maps, core_ids=list(range(N_CORES)),
                               trace=trace)
    global LAST_RESULTS
    LAST_RESULTS = res

    # T2[v] = MLP(c2e_weight[v]) assembled from the 8 windows
    t2 = np.empty((vocab, D), dtype=np.float32)
    for i in range(N_CORES):
        dense = res.results[i]["out"]                    # [128, rows] (k, r)
        lo = i * RANGE
        hi = min((i + 1) * RANGE, vocab)
        t2[lo:hi] = dense[:, lo - starts[i] : hi - starts[i]].T

    return t2[nodes]


# revision 17
# speedup vs baseline: 1.0443x; 1.0443x over previous
"""Trainium2 Bass kernel for the Context Encoder problem:

    ce  = c2e_weight[nodes]            # [N, 128] embedding gather
    h   = relu(ce @ w1.T + b1)         # [N, 128]
    out = relu(h @ w2.T + b2)          # [N, 128]

Strategy (8 NeuronCores, vocab-range sharding):
  200000 node ids over a 100000-row vocab saturate every vocab window,
  so transforming the table itself is less work than gathering per-node
  rows (and avoids the per-index DMA descriptor-generation cost that
  dominates any on-device gather).

  - The vocab is split into 8 fixed 12500-row ranges.  Core i streams
    its host-pre-transposed (d-major) table window [128, 12800]
    contiguously at full DMA bandwidth and computes
    T2 = relu(relu(win @ w1.T + b1) @ w2.T + b2) for every window row.
  - d-major input feeds mm1 directly (lhsT = w1.T stationary, window as
    the moving operand); mm2 keeps w2.T stationary.  Both layers' biases
    are per-partition, so relu+bias fuses into one ScalarE activation or
    one VectorE dual-op tensor_scalar; the two relus alternate between
    ACT and DVE for engine balance.  No PE transposes, no PSUM staging
    copies, no bias matmuls.
  - Results stay feature-major; the host transposes each window and maps
    node positions to rows (out = T2[nodes]) as the unshard step.
"""

import os
import sys

for _p in ("/opt/trn_rl_repo",):
    if _p not in sys.path:
        sys.path.insert(0, _p)

import numpy as np

import concourse.bass as bass
import concourse.mybir as mybir
import concourse.tile as tile
from concourse import bacc
from concourse.bass_utils import run_bass_kernel_spmd
from concourse.tile import TileContext

P = 128
D = 128
N_CORES = 8
VOCAB = 100000
RANGE = VOCAB // N_CORES   # 12500 vocab rows owned per core
BLOCKS = 100               # 12800 rows processed per core (128*100)
CHUNK_BLOCKS = 20          # rows DMA'd per input chunk (1.31 MB)
G = 4                      # blocks per compute super-tile (free dim 512)


def build_nc(blocks: int = BLOCKS, chunk_blocks: int = CHUNK_BLOCKS,
             g: int = G, use_f32r: bool = False):
    assert blocks % g == 0 and chunk_blocks % g == 0
    f32 = mybir.dt.float32
    nc = bacc.Bacc("TRN2", target_bir_lowering=False, debug=False,
                   num_devices=N_CORES)

    rows = blocks * P
    tsl_t = nc.dram_tensor("tslice", [P, rows], f32,
                           kind="ExternalInput").ap()
    w1t_t = nc.dram_tensor("w1t", [D, D], f32, kind="ExternalInput").ap()
    w2t_t = nc.dram_tensor("w2t", [D, D], f32, kind="ExternalInput").ap()
    b1_t = nc.dram_tensor("b1c", [P, 1], f32, kind="ExternalInput").ap()
    b2_t = nc.dram_tensor("b2c", [P, 1], f32, kind="ExternalInput").ap()
    out_t = nc.dram_tensor("out", [P, rows], f32,
                           kind="ExternalOutput").ap()

    fw = g * D  # super-tile free width (512)

    with TileContext(nc) as tc:
        with (
            tc.tile_pool(name="const", bufs=1) as cpool,
            tc.tile_pool(name="win", bufs=3) as gpool,
            tc.tile_pool(name="work", bufs=3) as wpool,
            tc.tile_pool(name="psum", bufs=3, space="PSUM") as ppool,
        ):
            w1t_sb = cpool.tile([D, D], f32, tag="w1t")
            nc.sync.dma_start(out=w1t_sb[:], in_=w1t_t[:])
            w2t_sb = cpool.tile([D, D], f32, tag="w2t")
            nc.sync.dma_start(out=w2t_sb[:], in_=w2t_t[:])
            b1_sb = cpool.tile([P, 1], f32, tag="b1")
            nc.sync.dma_start(out=b1_sb[:], in_=b1_t[:])
            b2_sb = cpool.tile([P, 1], f32, tag="b2")
            nc.sync.dma_start(out=b2_sb[:], in_=b2_t[:])

            def relu_bias(out_ap, in_ap, bias_sb, on_act: bool):
                if on_act:
                    nc.scalar.activation(out_ap, in_ap,
                                         mybir.ActivationFunctionType.Relu,
                                         bias=bias_sb[:, 0:1])
                else:
                    nc.vector.tensor_scalar(
                        out=out_ap, in0=in_ap, scalar1=bias_sb[:, 0:1],
                        scalar2=0.0, op0=mybir.AluOpType.add,
                        op1=mybir.AluOpType.max)

            def mmcast(ap):
                return ap.bitcast(mybir.dt.float32r) if use_f32r else ap

            # small first chunk so mm1 starts as early as possible
            chunks = [g] + [chunk_blocks] * ((blocks - g) // chunk_blocks)
            rem = blocks - sum(chunks)
            assert rem % g == 0
            if rem:
                chunks.append(rem)

            st = 0
            r0 = 0
            for cb in chunks:
                win = gpool.tile([P, chunk_blocks * D], f32, tag="win")
                nc.sync.dma_start(
                    out=win[:, : cb * D], in_=tsl_t[:, r0 : r0 + cb * P])
                for s in range(cb // g):
                    r0s = r0 + s * fw
                    ceT = win[:, s * fw : (s + 1) * fw]

                    h_ps = ppool.tile([P, fw], f32, tag="h")
                    nc.tensor.matmul(out=h_ps[:], lhsT=mmcast(w1t_sb[:]),
                                     rhs=mmcast(ceT), start=True, stop=True)
                    hT_sb = wpool.tile([P, fw], f32, tag="hT")
                    relu_bias(hT_sb[:], h_ps[:], b1_sb, on_act=(st % 2 == 0))

                    o_ps = ppool.tile([P, fw], f32, tag="o")
                    nc.tensor.matmul(out=o_ps[:], lhsT=mmcast(w2t_sb[:]),
                                     rhs=mmcast(hT_sb[:]), start=True,
                                     stop=True)
                    o_sb = wpool.tile([P, fw], f32, tag="o_sb")
                    relu_bias(o_sb[:], o_ps[:], b2_sb, on_act=(st % 2 == 1))
                    st += 1

                    nc.sync.dma_start(out=out_t[:, r0s : r0s + fw],
                                      in_=o_sb[:])
                r0 += cb * P

    nc.compile()
    return nc


_CACHED_NC = None
LAST_RESULTS = None


def _get_nc():
    global _CACHED_NC
    if _CACHED_NC is None:
        _CACHED_NC = build_nc(
            use_f32r=os.environ.get("BASS_KERNEL_F32R", "0") == "1")
    return _CACHED_NC


def kernel(nodes, c2e_weight, w1, b1, w2, b2):
    nodes = np.asarray(nodes).astype(np.int64)
    c2e_weight = np.asarray(c2e_weight, dtype=np.float32)
    w1 = np.asarray(w1, dtype=np.float32)
    b1 = np.asarray(b1, dtype=np.float32)
    w2 = np.asarray(w2, dtype=np.float32)
    b2 = np.asarray(b2, dtype=np.float32)

    vocab = c2e_weight.shape[0]
    assert vocab == VOCAB, vocab
    rows = BLOCKS * P  # 12800

    tableT = np.ascontiguousarray(c2e_weight.T)  # [128, VOCAB], d-major

    w1t = np.ascontiguousarray(w1.T)
    w2t = np.ascontiguousarray(w2.T)
    b1c = np.ascontiguousarray(b1.reshape(P, 1))
    b2c = np.ascontiguousarray(b2.reshape(P, 1))

    starts = []
    in_maps = []
    for i in range(N_CORES):
        start = min(i * RANGE, vocab - rows)
        starts.append(start)
        in_maps.append({
            "tslice": np.ascontiguousarray(tableT[:, start : start + rows]),
            "w1t": w1t,
            "w2t": w2t,
            "b1c": b1c,
            "b2c": b2c,
        })

    nc = _get_nc()
    trace = os.environ.get("BASS_KERNEL_TRACE") == "1"
    res = run_bass_kernel_spmd(nc, in_maps, core_ids=list(range(N_CORES)),
                               trace=trace)
    global LAST_RESULTS
    LAST_RESULTS = res

    # T2[v] = MLP(c2e_weight[v]) assembled from the 8 windows
    t2 = np.empty((vocab, D), dtype=np.float32)
    for i in range(N_CORES):
        dense = res.results[i]["out"]                    # [128, rows] (k, r)
        lo = i * RANGE
        hi = min((i + 1) * RANGE, vocab)
        t2[lo:hi] = dense[:, lo - starts[i] : hi - starts[i]].T

    return t2[nodes]
